# revision 29
# baseline (speedup 1.0000x reference)
"""Luong 'general' attention for TRN2, data-parallel over batch on 8 NeuronCores.

Math: energies[b,l] = hidden[b] . (W @ enc[l,b] + bias); out = softmax_l(energies).
Softmax is invariant to a per-row constant, so the bias term (hidden[b] . bias)
drops out exactly, and
  energies[b,l] = (hidden[b] @ W) . enc[l,b]  =  v[b] . enc[l,b]
so we compute v = hid @ W once (tiny), then a memory-bound batched dot over
encoder_outputs (512 MB), then a softmax over l.

Sharding: batch dim B=32 split 4-per-core across 8 cores. encoder_outputs is
pre-transposed on the host to [B, H, L] so each core's DMA streams [128h, L]
tiles with H on partitions, making the dot product a plain TensorE matmul
(contraction over partitions).

Precision modes for the big matmul:
  - "bf16x2" (default): enc and v are split into bf16 hi+lo pairs on the
    host/device; three bf16 matmul streams (vh.xh + vl.xh + vh.xl) recover
    ~17 mantissa bits. Output absmax error ~1e-4 vs fp32 reference.
  - "float32r": PE reduced-precision fp32 mode (~12-bit), 1 stream.
  - "float32": exact fp32 matmul (4x slower PE, still correct output).
"""

import numpy as np
from contextlib import ExitStack

import ml_dtypes
import concourse.bass as bass
import concourse.tile as tile
import concourse.mybir as mybir
from concourse import bacc
from concourse.bass_utils import run_bass_kernel_spmd

B, L, H = 32, 4096, 1024
NCORES = 8
BP = B // NCORES          # 4 batch rows per core
P = 128
HO = H // P               # 8 h-chunks
NJ = 512                  # matmul free-dim tile (one PSUM bank of fp32)

MODE = "f16x8"

_cache = {}


def _softmax_row(nc, tc, work, small, e_src, row, out_row):
    """softmax along free dim: max/exp read e_src (SBUF or PSUM), result lands
    in the SBUF tile `row` and is DMA'd to out_row."""
    f32 = mybir.dt.float32
    mx = small.tile([1, 1], f32, tag="mx")
    nc.vector.reduce_max(mx[:], e_src[:], axis=mybir.AxisListType.X)
    nmx = small.tile([1, 1], f32, tag="nmx")
    nc.vector.tensor_scalar_mul(nmx[:], mx[:], -1.0)
    sm = small.tile([1, 1], f32, tag="sm")
    nc.scalar.activation(
        row[:],
        e_src[:],
        mybir.ActivationFunctionType.Exp,
        bias=nmx[:],
        scale=1.0,
        accum_out=sm[:],
    )
    rv = small.tile([1, 1], f32, tag="rv")
    nc.vector.reciprocal(rv[:], sm[:])
    nc.vector.tensor_scalar_mul(row[:], row[:], rv[:])
    nc.sync.dma_start(out_row, row[:])


def _compute_vT(nc, tc, const, w, hidT):
    """vT[h, b] = sum_g W[g, h] hid[b, g], fp32, as [P, HO, BP] in SBUF."""
    f32 = mybir.dt.float32
    w_sb = const.tile([P, HO, H], f32)
    # issue on the ACT HWDGE ring so the big enc stream on the SP ring
    # isn't queued behind this 4MB load
    nc.scalar.dma_start(w_sb[:], w.rearrange("(go gp) h -> gp go h", gp=P))
    hidT_sb = const.tile([P, HO, BP], f32)
    nc.scalar.dma_start(hidT_sb[:], hidT.rearrange("(go gp) b -> gp go b", gp=P))

    vT_sb = const.tile([P, HO, BP], f32)
    with tc.tile_pool(name="psv", bufs=2, space="PSUM") as psv:
        for ho in range(HO):
            pv = psv.tile([P, BP], f32)
            for go in range(HO):
                nc.tensor.matmul(
                    pv[:],
                    w_sb[:, go, ho * P : (ho + 1) * P],
                    hidT_sb[:, go, :],
                    start=(go == 0),
                    stop=(go == HO - 1),
                )
            nc.scalar.copy(vT_sb[:, ho, :], pv[:])
    return vT_sb


def _build(mode, repeat=1, nho=2, bufs=None, internal_enc=False, ring_alt=False, lite=False):
    if mode == "f16x8lite":
        mode, lite = "f16x8", True
    if bufs is None:
        bufs = 4 if mode == "f16x8" else 3
    f32 = mybir.dt.float32
    bf16 = mybir.dt.bfloat16
    nc = bacc.Bacc(
        "TRN2", target_bir_lowering=False, debug=False, num_devices=NCORES
    )
    hidT = nc.dram_tensor("hidT", [H, BP], f32, kind="ExternalInput").ap()
    w = nc.dram_tensor("w", [H, H], f32, kind="ExternalInput").ap()
    out = nc.dram_tensor("out", [BP, L], f32, kind="ExternalOutput").ap()
    f16 = mybir.dt.float16
    f8 = mybir.dt.float8e5
    if mode == "f16x8":
        enc_shapes = {"encH": ([BP, HO, P, L], f16), "encL": ([BP, HO, P, L], f8)}
    elif mode in ("bf16x2", "dmaonly"):
        enc_shapes = {"encT": ([BP, HO, P, 2, L], bf16)}
    else:
        enc_shapes = {"encT": ([BP, H, L], f32)}
    encs = {}
    if not internal_enc:
        for nm, (shp, dt) in enc_shapes.items():
            encs[nm] = nc.dram_tensor(nm, shp, dt, kind="ExternalInput").ap()
    encT = encs.get("encT")
    mm_dt = {"float32": f32, "float32r": mybir.dt.float32r}.get(mode)

    with tile.TileContext(nc) as tc:
        with ExitStack() as ctx:
            const = ctx.enter_context(tc.tile_pool(name="const", bufs=1))
            encp = ctx.enter_context(tc.tile_pool(name="encp", bufs=bufs))
            work = ctx.enter_context(tc.tile_pool(name="work", bufs=2))
            small = ctx.enter_context(tc.tile_pool(name="small", bufs=8))

            if internal_enc:
                # timing-only variant: enc lives in device DRAM (zero-filled),
                # so per-call host<->device traffic is just w/hidT
                dramp = ctx.enter_context(
                    tc.tile_pool(name="dram", bufs=1, space="DRAM")
                )
                for nm, (shp, dt) in enc_shapes.items():
                    encs[nm] = dramp.tile(shp, dt, name=f"enc_{nm}", tag=f"enc_{nm}")
                    zt = const.tile([P, L], dt, tag=f"z_{nm}")
                    nc.vector.memset(zt[:], 0.0)
                    t = encs[nm]
                    for b in range(BP):
                        for x in range(HO):
                            if mode == "f16x8":
                                nc.sync.dma_start(t[b, x], zt[:])
                            elif mode in ("bf16x2", "dmaonly"):
                                for two in range(2):
                                    nc.sync.dma_start(t[b, x, :, two, :], zt[:])
                            else:
                                nc.sync.dma_start(t[b, x * P : (x + 1) * P, :], zt[:])
                encT = encs.get("encT")

            vT_f32 = _compute_vT(nc, tc, const, w, hidT)

            if mode == "f16x8":
                # v = vh(f16) + vl(f16); lo-stream weights are e5m2(vh)
                vh = const.tile([P, HO, BP], f16)
                nc.scalar.copy(vh[:], vT_f32[:])
                vh_f32 = const.tile([P, HO, BP], f32)
                nc.vector.tensor_copy(vh_f32[:], vh[:])
                vd = const.tile([P, HO, BP], f32)
                nc.vector.tensor_tensor(
                    vd[:], vT_f32[:], vh_f32[:], mybir.AluOpType.subtract
                )
                vl = const.tile([P, HO, BP], f16)
                nc.vector.tensor_copy(vl[:], vd[:])
                vh8 = const.tile([P, HO, BP], f8)
                nc.scalar.copy(vh8[:], vh_f32[:])
                vT_sb = None
            elif mode == "bf16x2":
                # split vT into bf16 hi + lo (hi = bf16(v), lo = bf16(v - hi))
                vh = const.tile([P, HO, BP], bf16)
                nc.scalar.copy(vh[:], vT_f32[:])
                vh_f32 = const.tile([P, HO, BP], f32)
                nc.vector.tensor_copy(vh_f32[:], vh[:])
                vd = const.tile([P, HO, BP], f32)
                nc.vector.tensor_tensor(
                    vd[:], vT_f32[:], vh_f32[:], mybir.AluOpType.subtract
                )
                vl = const.tile([P, HO, BP], bf16)
                nc.vector.tensor_copy(vl[:], vd[:])
                vT_sb = None
            elif mode == "dmaonly":
                vT_sb = None
            else:
                if mm_dt != f32:
                    vT_sb = const.tile([P, HO, BP], mm_dt)
                    nc.scalar.copy(vT_sb[:], vT_f32[:])
                else:
                    vT_sb = vT_f32

            if mode == "dmaonly":
                # pure-stream probe: load everything, emit a dummy output
                for b in [bb % BP for bb in range(BP * repeat)]:
                    for ho in range(0, HO, nho):
                        et = encp.tile([P, nho, 2, L], bf16, tag="enc")
                        nc.sync.dma_start(
                            et[:],
                            encT[b, ho : ho + nho].rearrange("o p two l -> p o two l"),
                        )
                        if ho + nho >= HO:
                            ot = work.tile([1, L], f32, tag="ot")
                            nc.vector.tensor_copy(ot[:], et[:1, 0, 0, :])
                            nc.sync.dma_start(out[b : b + 1, :], ot[:])
                bp_iters = []
            else:
                bp_iters = [bb % BP for bb in range(BP * repeat)]

            pse = ctx.enter_context(tc.tile_pool(name="pse", bufs=1, space="PSUM"))
            for bi, b in enumerate(bp_iters):
                pe = pse.tile([1, L], f32, tag="pe")
                for ho0 in range(0, HO, nho):
                    if mode == "f16x8":
                        eth = encp.tile([P, nho, L], f16, tag="ench")
                        etl = encp.tile([P, nho, L], f8, tag="encl")
                        nc.sync.dma_start(
                            eth[:],
                            encs["encH"][b, ho0 : ho0 + nho].rearrange(
                                "o p l -> p o l"
                            ),
                        )
                        nc.scalar.dma_start(
                            etl[:],
                            encs["encL"][b, ho0 : ho0 + nho].rearrange(
                                "o p l -> p o l"
                            ),
                        )
                        for o in range(nho):
                            ho = ho0 + o
                            # weight-stationary: run each stream's 8 chunks
                            # back-to-back so the PE swaps weights 3x per
                            # h-chunk instead of 24x
                            streams = [(vh, eth, "s1")]
                            if not lite:
                                streams.append((vl, eth, "s2"))
                            streams.append((vh8, etl, "s3"))
                            for si, (vw, rhs_t, _) in enumerate(streams):
                                for j in range(L // NJ):
                                    js = slice(j * NJ, (j + 1) * NJ)
                                    nc.tensor.matmul(
                                        pe[:, js], vw[:, ho, b : b + 1],
                                        rhs_t[:, o, js],
                                        start=(ho == 0 and si == 0),
                                        stop=(ho == HO - 1 and si == len(streams) - 1),
                                    )
                    elif mode == "bf16x2":
                        et = encp.tile([P, nho, 2, L], bf16, tag="enc")
                        eng = (
                            nc.scalar
                            if ring_alt and (ho0 // nho) % 2 == 1
                            else nc.sync
                        )
                        eng.dma_start(
                            et[:],
                            encT[b, ho0 : ho0 + nho].rearrange(
                                "o p two l -> p o two l"
                            ),
                        )
                        for o in range(nho):
                            ho = ho0 + o
                            eh, el = et[:, o, 0, :], et[:, o, 1, :]
                            for j in range(L // NJ):
                                js = slice(j * NJ, (j + 1) * NJ)
                                nc.tensor.matmul(
                                    pe[:, js], vh[:, ho, b : b + 1], eh[:, js],
                                    start=(ho == 0), stop=False,
                                )
                                nc.tensor.matmul(
                                    pe[:, js], vl[:, ho, b : b + 1], eh[:, js],
                                    start=False, stop=False,
                                )
                                nc.tensor.matmul(
                                    pe[:, js], vh[:, ho, b : b + 1], el[:, js],
                                    start=False, stop=(ho == HO - 1),
                                )
                    else:
                        ho = ho0
                        et = encp.tile([P, L], mm_dt, tag="enc")
                        src = encT[b, ho * P : (ho + 1) * P, :]
                        nc.sync.dma_start(
                            et[:], src.bitcast(mm_dt) if mm_dt != f32 else src
                        )
                        for j in range(L // NJ):
                            js = slice(j * NJ, (j + 1) * NJ)
                            nc.tensor.matmul(
                                pe[:, js], vT_sb[:, ho, b : b + 1], et[:, js],
                                start=(ho == 0), stop=(ho == HO - 1),
                            )
                last = bi == len(bp_iters) - 1
                if not last:
                    # free PSUM quickly so the next b's matmuls can start
                    e_src = work.tile([1, L], f32, tag="row")
                    nc.scalar.copy(e_src[:], pe[:])
                else:
                    e_src = pe  # last row: softmax max/exp read PSUM directly
                row = work.tile([1, L], f32, tag="row")
                _softmax_row(nc, tc, work, small, e_src, row, out[b : b + 1, :])

    nc.finalize()
    return nc


def _prep_encT(encoder_outputs, mode):
    if mode == "f16x8lite":
        mode = "f16x8"
    encT = np.ascontiguousarray(encoder_outputs.transpose(1, 2, 0))  # [B, H, L]
    if mode == "f16x8":
        hi = encT.astype(np.float16)
        lo = (encT - hi.astype(np.float32)).astype(ml_dtypes.float8_e5m2)
        return {
            "encH": hi.reshape(B, HO, P, L),
            "encL": lo.reshape(B, HO, P, L),
        }
    if mode not in ("bf16x2", "dmaonly"):
        return {"encT": encT}
    bf = ml_dtypes.bfloat16
    hi = encT.astype(bf)
    lo = (encT - hi.astype(np.float32)).astype(bf)
    # [B, HO, P, 2, L]
    packed = np.empty((B, HO, P, 2, L), dtype=bf)
    packed[:, :, :, 0] = hi.reshape(B, HO, P, L)
    packed[:, :, :, 1] = lo.reshape(B, HO, P, L)
    return {"encT": packed}


def make_in_maps(hidden, encoder_outputs, W, mode=None):
    mode = mode or MODE
    hidden = np.asarray(hidden, dtype=np.float32)
    encoder_outputs = np.asarray(encoder_outputs, dtype=np.float32)
    W = np.asarray(W, dtype=np.float32)
    encs = _prep_encT(encoder_outputs, mode)
    hidT_full = np.ascontiguousarray(hidden[0].T)  # [H, B]
    in_maps = []
    for c in range(NCORES):
        m = {nm: a[c * BP : (c + 1) * BP] for nm, a in encs.items()}
        m["hidT"] = np.ascontiguousarray(hidT_full[:, c * BP : (c + 1) * BP])
        m["w"] = W
        in_maps.append(m)
    return in_maps


def kernel(hidden, encoder_outputs, W, b, _trace=False):
    if MODE not in _cache:
        _cache[MODE] = _build(MODE)
    nc = _cache[MODE]
    in_maps = make_in_maps(hidden, encoder_outputs, W, MODE)
    res = run_bass_kernel_spmd(
        nc, in_maps, core_ids=list(range(NCORES)), trace=_trace
    )
    out = np.empty((B, 1, L), dtype=np.float32)
    for c in range(NCORES):
        out[c * BP : (c + 1) * BP, 0, :] = res.results[c]["out"]
    if _trace:
        kernel.last_result = res
    return out


# revision 32
# speedup vs baseline: 1.1656x; 1.1656x over previous
"""Luong 'general' attention for TRN2, data-parallel over batch on 8 NeuronCores.

Math: energies[b,l] = hidden[b] . (W @ enc[l,b] + bias); out = softmax_l(energies).
Softmax is invariant to a per-row constant, so the bias term (hidden[b] . bias)
drops out exactly, and
  energies[b,l] = (hidden[b] @ W) . enc[l,b]  =  v[b] . enc[l,b]
so we compute v = hid @ W once (tiny), then a memory-bound batched dot over
encoder_outputs (512 MB), then a softmax over l.

Sharding: batch dim B=32 split 4-per-core across 8 cores. encoder_outputs is
pre-transposed on the host to [B, H, L] so each core's DMA streams [128h, L]
tiles with H on partitions, making the dot product a plain TensorE matmul
(contraction over partitions).

Precision modes for the big matmul:
  - "bf16x2" (default): enc and v are split into bf16 hi+lo pairs on the
    host/device; three bf16 matmul streams (vh.xh + vl.xh + vh.xl) recover
    ~17 mantissa bits. Output absmax error ~1e-4 vs fp32 reference.
  - "float32r": PE reduced-precision fp32 mode (~12-bit), 1 stream.
  - "float32": exact fp32 matmul (4x slower PE, still correct output).
"""

import numpy as np
from contextlib import ExitStack

import ml_dtypes
import concourse.bass as bass
import concourse.tile as tile
import concourse.mybir as mybir
from concourse import bacc
from concourse.bass_utils import run_bass_kernel_spmd

B, L, H = 32, 4096, 1024
NCORES = 8
BP = B // NCORES          # 4 batch rows per core
P = 128
HO = H // P               # 8 h-chunks
NJ = 512                  # matmul free-dim tile (one PSUM bank of fp32)

MODE = "f16x8"

_cache = {}


def _softmax_row(nc, tc, work, small, e_src, row, out_row):
    """softmax along free dim: max/exp read e_src (SBUF or PSUM), result lands
    in the SBUF tile `row` and is DMA'd to out_row."""
    f32 = mybir.dt.float32
    mx = small.tile([1, 1], f32, tag="mx")
    nc.vector.reduce_max(mx[:], e_src[:], axis=mybir.AxisListType.X)
    nmx = small.tile([1, 1], f32, tag="nmx")
    nc.vector.tensor_scalar_mul(nmx[:], mx[:], -1.0)
    sm = small.tile([1, 1], f32, tag="sm")
    nc.scalar.activation(
        row[:],
        e_src[:],
        mybir.ActivationFunctionType.Exp,
        bias=nmx[:],
        scale=1.0,
        accum_out=sm[:],
    )
    rv = small.tile([1, 1], f32, tag="rv")
    nc.vector.reciprocal(rv[:], sm[:])
    nc.vector.tensor_scalar_mul(row[:], row[:], rv[:])
    nc.sync.dma_start(out_row, row[:])


def _compute_vT(nc, tc, const, w, hidT):
    """vT[h, b] = sum_g W[g, h] hid[b, g], fp32, as [P, HO, BP] in SBUF."""
    f32 = mybir.dt.float32
    w_sb = const.tile([P, HO, H], f32)
    # issue on the ACT HWDGE ring so the big enc stream on the SP ring
    # isn't queued behind this 4MB load
    nc.scalar.dma_start(w_sb[:], w.rearrange("(go gp) h -> gp go h", gp=P))
    hidT_sb = const.tile([P, HO, BP], f32)
    nc.scalar.dma_start(hidT_sb[:], hidT.rearrange("(go gp) b -> gp go b", gp=P))

    vT_sb = const.tile([P, HO, BP], f32)
    with tc.tile_pool(name="psv", bufs=2, space="PSUM") as psv:
        for ho in range(HO):
            pv = psv.tile([P, BP], f32)
            for go in range(HO):
                nc.tensor.matmul(
                    pv[:],
                    w_sb[:, go, ho * P : (ho + 1) * P],
                    hidT_sb[:, go, :],
                    start=(go == 0),
                    stop=(go == HO - 1),
                )
            nc.scalar.copy(vT_sb[:, ho, :], pv[:])
    return vT_sb


def _build(mode, repeat=1, nho=2, bufs=None, internal_enc=False, ring_alt=False, lite=False):
    if mode == "f16x8lite":
        mode, lite = "f16x8", True
    if bufs is None:
        bufs = 4 if mode == "f16x8" else 3
    f32 = mybir.dt.float32
    bf16 = mybir.dt.bfloat16
    nc = bacc.Bacc(
        "TRN2", target_bir_lowering=False, debug=False, num_devices=NCORES
    )
    hidT = nc.dram_tensor("hidT", [H, BP], f32, kind="ExternalInput").ap()
    w = nc.dram_tensor("w", [H, H], f32, kind="ExternalInput").ap()
    out = nc.dram_tensor("out", [BP, L], f32, kind="ExternalOutput").ap()
    f16 = mybir.dt.float16
    f8 = mybir.dt.float8e5
    if mode == "f16x8":
        enc_shapes = {"encH": ([BP, HO, P, L], f16), "encL": ([BP, HO, P, L], f8)}
    elif mode in ("bf16x2", "dmaonly"):
        enc_shapes = {"encT": ([BP, HO, P, 2, L], bf16)}
    else:
        enc_shapes = {"encT": ([BP, H, L], f32)}
    encs = {}
    if not internal_enc:
        for nm, (shp, dt) in enc_shapes.items():
            encs[nm] = nc.dram_tensor(nm, shp, dt, kind="ExternalInput").ap()
    encT = encs.get("encT")
    mm_dt = {"float32": f32, "float32r": mybir.dt.float32r}.get(mode)

    with tile.TileContext(nc) as tc:
        with ExitStack() as ctx:
            const = ctx.enter_context(tc.tile_pool(name="const", bufs=1))
            encp = ctx.enter_context(tc.tile_pool(name="encp", bufs=bufs))
            work = ctx.enter_context(tc.tile_pool(name="work", bufs=2))
            small = ctx.enter_context(tc.tile_pool(name="small", bufs=8))

            if internal_enc:
                # timing-only variant: enc lives in device DRAM (zero-filled),
                # so per-call host<->device traffic is just w/hidT
                dramp = ctx.enter_context(
                    tc.tile_pool(name="dram", bufs=1, space="DRAM")
                )
                for nm, (shp, dt) in enc_shapes.items():
                    encs[nm] = dramp.tile(shp, dt, name=f"enc_{nm}", tag=f"enc_{nm}")
                    zt = const.tile([P, L], dt, tag=f"z_{nm}")
                    nc.vector.memset(zt[:], 0.0)
                    t = encs[nm]
                    for b in range(BP):
                        for x in range(HO):
                            if mode == "f16x8":
                                nc.sync.dma_start(t[b, x], zt[:])
                            elif mode in ("bf16x2", "dmaonly"):
                                for two in range(2):
                                    nc.sync.dma_start(t[b, x, :, two, :], zt[:])
                            else:
                                nc.sync.dma_start(t[b, x * P : (x + 1) * P, :], zt[:])
                encT = encs.get("encT")

            vT_f32 = _compute_vT(nc, tc, const, w, hidT)

            if mode == "f16x8":
                # v = vh(f16) + vl(f16); lo-stream weights are e5m2(vh)
                vh = const.tile([P, HO, BP], f16)
                nc.scalar.copy(vh[:], vT_f32[:])
                vh_f32 = const.tile([P, HO, BP], f32)
                nc.vector.tensor_copy(vh_f32[:], vh[:])
                vd = const.tile([P, HO, BP], f32)
                nc.vector.tensor_tensor(
                    vd[:], vT_f32[:], vh_f32[:], mybir.AluOpType.subtract
                )
                vl = const.tile([P, HO, BP], f16)
                nc.vector.tensor_copy(vl[:], vd[:])
                vh8 = const.tile([P, HO, BP], f8)
                nc.scalar.copy(vh8[:], vh_f32[:])
                vT_sb = None
            elif mode == "bf16x2":
                # split vT into bf16 hi + lo (hi = bf16(v), lo = bf16(v - hi))
                vh = const.tile([P, HO, BP], bf16)
                nc.scalar.copy(vh[:], vT_f32[:])
                vh_f32 = const.tile([P, HO, BP], f32)
                nc.vector.tensor_copy(vh_f32[:], vh[:])
                vd = const.tile([P, HO, BP], f32)
                nc.vector.tensor_tensor(
                    vd[:], vT_f32[:], vh_f32[:], mybir.AluOpType.subtract
                )
                vl = const.tile([P, HO, BP], bf16)
                nc.vector.tensor_copy(vl[:], vd[:])
                vT_sb = None
            elif mode == "dmaonly":
                vT_sb = None
            else:
                if mm_dt != f32:
                    vT_sb = const.tile([P, HO, BP], mm_dt)
                    nc.scalar.copy(vT_sb[:], vT_f32[:])
                else:
                    vT_sb = vT_f32

            if mode == "dmaonly":
                # pure-stream probe: load everything, emit a dummy output
                for b in [bb % BP for bb in range(BP * repeat)]:
                    for ho in range(0, HO, nho):
                        et = encp.tile([P, nho, 2, L], bf16, tag="enc")
                        nc.sync.dma_start(
                            et[:],
                            encT[b, ho : ho + nho].rearrange("o p two l -> p o two l"),
                        )
                        if ho + nho >= HO:
                            ot = work.tile([1, L], f32, tag="ot")
                            nc.vector.tensor_copy(ot[:], et[:1, 0, 0, :])
                            nc.sync.dma_start(out[b : b + 1, :], ot[:])
                bp_iters = []
            else:
                bp_iters = [bb % BP for bb in range(BP * repeat)]

            pse = ctx.enter_context(tc.tile_pool(name="pse", bufs=1, space="PSUM"))
            for bi, b in enumerate(bp_iters):
                pe = pse.tile([33, L], f32, tag="pe")
                for ho0 in range(0, HO, nho):
                    if mode == "f16x8":
                        eth = encp.tile([P, nho, L], f16, tag="ench")
                        etl = encp.tile([P, nho, L], f8, tag="encl")
                        nc.sync.dma_start(
                            eth[:],
                            encs["encH"][b, ho0 : ho0 + nho].rearrange(
                                "o p l -> p o l"
                            ),
                        )
                        nc.scalar.dma_start(
                            etl[:],
                            encs["encL"][b, ho0 : ho0 + nho].rearrange(
                                "o p l -> p o l"
                            ),
                        )
                        for o in range(nho):
                            ho = ho0 + o
                            # weight-stationary: run each stream's 8 chunks
                            # back-to-back so the PE swaps weights 3x per
                            # h-chunk instead of 24x
                            for j in range(L // NJ):
                                js = slice(j * NJ, (j + 1) * NJ)
                                # vh and vl share one xh stream: vl runs in
                                # col-group 32 concurrently with vh
                                nc.tensor.matmul(
                                    pe[0:1, js], vh[:, ho, b : b + 1],
                                    eth[:, o, js],
                                    start=(ho == 0), stop=False,
                                )
                                if not lite:
                                    nc.tensor.matmul(
                                        pe[32:33, js], vl[:, ho, b : b + 1],
                                        eth[:, o, js],
                                        start=(ho == 0), stop=(ho == HO - 1),
                                        tile_position=(0, 32),
                                    )
                                nc.tensor.matmul(
                                    pe[0:1, js], vh8[:, ho, b : b + 1],
                                    etl[:, o, js],
                                    start=False, stop=(ho == HO - 1),
                                )
                    elif mode == "bf16x2":
                        et = encp.tile([P, nho, 2, L], bf16, tag="enc")
                        eng = (
                            nc.scalar
                            if ring_alt and (ho0 // nho) % 2 == 1
                            else nc.sync
                        )
                        eng.dma_start(
                            et[:],
                            encT[b, ho0 : ho0 + nho].rearrange(
                                "o p two l -> p o two l"
                            ),
                        )
                        for o in range(nho):
                            ho = ho0 + o
                            eh, el = et[:, o, 0, :], et[:, o, 1, :]
                            for j in range(L // NJ):
                                js = slice(j * NJ, (j + 1) * NJ)
                                nc.tensor.matmul(
                                    pe[:, js], vh[:, ho, b : b + 1], eh[:, js],
                                    start=(ho == 0), stop=False,
                                )
                                nc.tensor.matmul(
                                    pe[:, js], vl[:, ho, b : b + 1], eh[:, js],
                                    start=False, stop=False,
                                )
                                nc.tensor.matmul(
                                    pe[:, js], vh[:, ho, b : b + 1], el[:, js],
                                    start=False, stop=(ho == HO - 1),
                                )
                    else:
                        ho = ho0
                        et = encp.tile([P, L], mm_dt, tag="enc")
                        src = encT[b, ho * P : (ho + 1) * P, :]
                        nc.sync.dma_start(
                            et[:], src.bitcast(mm_dt) if mm_dt != f32 else src
                        )
                        for j in range(L // NJ):
                            js = slice(j * NJ, (j + 1) * NJ)
                            nc.tensor.matmul(
                                pe[:, js], vT_sb[:, ho, b : b + 1], et[:, js],
                                start=(ho == 0), stop=(ho == HO - 1),
                            )
                e_src = work.tile([1, L], f32, tag="row")
                nc.scalar.copy(e_src[:], pe[0:1, :])
                if mode == "f16x8" and not lite:
                    # e = row0 (vh.xh + vh8.xl) + row32 (vl.xh); one PSUM
                    # operand per instruction (DVE has a single PSUM port)
                    nc.vector.tensor_tensor(
                        e_src[:], e_src[:], pe[32:33, :], mybir.AluOpType.add
                    )
                row = work.tile([1, L], f32, tag="row")
                _softmax_row(nc, tc, work, small, e_src, row, out[b : b + 1, :])

    nc.finalize()
    return nc


def _prep_encT(encoder_outputs, mode):
    if mode == "f16x8lite":
        mode = "f16x8"
    encT = np.ascontiguousarray(encoder_outputs.transpose(1, 2, 0))  # [B, H, L]
    if mode == "f16x8":
        hi = encT.astype(np.float16)
        lo = (encT - hi.astype(np.float32)).astype(ml_dtypes.float8_e5m2)
        return {
            "encH": hi.reshape(B, HO, P, L),
            "encL": lo.reshape(B, HO, P, L),
        }
    if mode not in ("bf16x2", "dmaonly"):
        return {"encT": encT}
    bf = ml_dtypes.bfloat16
    hi = encT.astype(bf)
    lo = (encT - hi.astype(np.float32)).astype(bf)
    # [B, HO, P, 2, L]
    packed = np.empty((B, HO, P, 2, L), dtype=bf)
    packed[:, :, :, 0] = hi.reshape(B, HO, P, L)
    packed[:, :, :, 1] = lo.reshape(B, HO, P, L)
    return {"encT": packed}


def make_in_maps(hidden, encoder_outputs, W, mode=None):
    mode = mode or MODE
    hidden = np.asarray(hidden, dtype=np.float32)
    encoder_outputs = np.asarray(encoder_outputs, dtype=np.float32)
    W = np.asarray(W, dtype=np.float32)
    encs = _prep_encT(encoder_outputs, mode)
    hidT_full = np.ascontiguousarray(hidden[0].T)  # [H, B]
    in_maps = []
    for c in range(NCORES):
        m = {nm: a[c * BP : (c + 1) * BP] for nm, a in encs.items()}
        m["hidT"] = np.ascontiguousarray(hidT_full[:, c * BP : (c + 1) * BP])
        m["w"] = W
        in_maps.append(m)
    return in_maps


def kernel(hidden, encoder_outputs, W, b, _trace=False):
    if MODE not in _cache:
        _cache[MODE] = _build(MODE)
    nc = _cache[MODE]
    in_maps = make_in_maps(hidden, encoder_outputs, W, MODE)
    res = run_bass_kernel_spmd(
        nc, in_maps, core_ids=list(range(NCORES)), trace=_trace
    )
    out = np.empty((B, 1, L), dtype=np.float32)
    for c in range(NCORES):
        out[c * BP : (c + 1) * BP, 0, :] = res.results[c]["out"]
    if _trace:
        kernel.last_result = res
    return out
